# revision 2
# baseline (speedup 1.0000x reference)
"""AgentAttention distributed over 8 NeuronCores, data-parallel over batch.

Full inputs in, full output out. B=16 is split 2-per-core across the 8
cores; all weights are replicated. The per-head bias maps (bilinear
7x7 -> 56x56 upsampling of an_bias/na_bias plus the scalar head biases)
depend only on weights, so they are precomputed on host in numpy and fed
to the device kernel as ordinary inputs.
"""

import numpy as np
import jax
import jax.numpy as jnp

B, N, C = 16, 3136, 512
H = W = 56
HEADS, AGENT, POOL = 8, 49, 7
D = C // HEADS
SCALE = D ** -0.5
NDEV = 8
BPD = B // NDEV  # batches per device


def _bilin_matrix(n_out=56, n_in=7):
    # Half-pixel bilinear upsample matrix; edge renormalization of the
    # triangle kernel is equivalent to clamping the sample coordinate.
    R = np.zeros((n_out, n_in), np.float32)
    for i in range(n_out):
        s = (i + 0.5) * n_in / n_out - 0.5
        s = min(max(s, 0.0), float(n_in - 1))
        j0 = int(np.floor(s))
        j1 = min(j0 + 1, n_in - 1)
        f = s - j0
        R[i, j0] += 1.0 - f
        if j1 != j0:
            R[i, j1] += f
    return R


_R = _bilin_matrix()  # (56, 7)


def _device_model(x, q_w, kv_w, proj_w, proj_b, dwc_w9, dwc_b,
                  bias_ak, bias_qa):
    # x: (BPD, N, C) on one core
    b = x.shape[0]
    q = x @ q_w                                   # (b,n,c)
    kv = x @ kv_w                                 # (b,n,2c)
    k = kv[:, :, :C]
    v = kv[:, :, C:]

    # exact 8x8 mean pooling of q -> agent tokens
    qc = q.reshape(b, POOL, H // POOL, POOL, W // POOL, C)
    agent = qc.mean(axis=(2, 4)).reshape(b, AGENT, C)          # (b,49,c)

    q4 = q.reshape(b, N, HEADS, D).transpose(0, 2, 1, 3)        # (b,h,n,d)
    k4 = k.reshape(b, N, HEADS, D).transpose(0, 2, 1, 3)
    v4 = v.reshape(b, N, HEADS, D).transpose(0, 2, 1, 3)
    a4 = agent.reshape(b, AGENT, HEADS, D).transpose(0, 2, 1, 3)

    # Stage 1: agent <-> kv
    s1 = jnp.einsum('bhad,bhnd->bhan', a4 * SCALE, k4) + bias_ak[None]
    agent_attn = jax.nn.softmax(s1, axis=-1)
    agent_v = jnp.einsum('bhan,bhnd->bhad', agent_attn, v4)     # (b,h,49,d)

    # Stage 2: query <-> agent
    s2 = jnp.einsum('bhnd,bhad->bhna', q4 * SCALE, a4) + bias_qa[None]
    q_attn = jax.nn.softmax(s2, axis=-1)
    out = jnp.einsum('bhna,bhad->bhnd', q_attn, agent_v)
    out = out.transpose(0, 2, 1, 3).reshape(b, N, C)

    # depthwise 3x3 SAME conv on v, channel-last via 9 shifted adds
    v_img = v.reshape(b, H, W, C)
    vp = jnp.pad(v_img, ((0, 0), (1, 1), (1, 1), (0, 0)))
    acc = dwc_b[None, None, None, :]
    for di in range(3):
        for dj in range(3):
            acc = acc + vp[:, di:di + H, dj:dj + W, :] * dwc_w9[di, dj][None, None, None, :]
    dwc = acc.reshape(b, N, C)

    return (out + dwc) @ proj_w + proj_b


_PMAPPED = None


def _get_pmapped():
    global _PMAPPED
    if _PMAPPED is None:
        _PMAPPED = jax.pmap(
            _device_model,
            in_axes=(0,) + (None,) * 8,
            devices=jax.devices()[:NDEV],
        )
    return _PMAPPED


def kernel(x, q_w, kv_w, proj_w, proj_b, dwc_w, dwc_b,
           an_bias, na_bias, ah_bias, aw_bias, ha_bias, wa_bias):
    x = np.asarray(x, np.float32)

    # Host precompute of the per-head bias maps (weights-only, tiny).
    pb1 = np.einsum('hapq,Pp,Qq->haPQ', np.asarray(an_bias, np.float32),
                    _R, _R).reshape(HEADS, AGENT, N)
    pb2 = (np.asarray(ah_bias)[0, :, :, 0] + np.asarray(aw_bias)[0, :, :, 0])
    bias_ak = (pb1 + pb2[:, :, None]).astype(np.float32)        # (h,49,n)

    ab1 = np.einsum('hapq,Pp,Qq->haPQ', np.asarray(na_bias, np.float32),
                    _R, _R).reshape(HEADS, AGENT, N).transpose(0, 2, 1)
    ab2 = (np.asarray(ha_bias)[0, :, :, 0] + np.asarray(wa_bias)[0, :, :, 0])
    bias_qa = (ab1 + ab2[:, None, :]).astype(np.float32)        # (h,n,49)

    dwc_w9 = np.asarray(dwc_w, np.float32)[:, 0].transpose(1, 2, 0).copy()  # (3,3,C)

    xs = x.reshape(NDEV, BPD, N, C)
    fn = _get_pmapped()
    y = fn(xs, jnp.asarray(q_w), jnp.asarray(kv_w), jnp.asarray(proj_w),
           jnp.asarray(proj_b), jnp.asarray(dwc_w9), jnp.asarray(dwc_b),
           jnp.asarray(bias_ak), jnp.asarray(bias_qa))
    return np.asarray(y).reshape(B, N, C).astype(np.float32)
